# revision 32
# baseline (speedup 1.0000x reference)
"""Trainium2 Bass kernel for nn_DiscreteLoss (data-parallel over batch).

Contract: kernel(**inputs) takes the FULL unsharded inputs (B=64) and
returns the FULL scalar loss.  Internally the batch dim is sharded over
8 NeuronCores (8 batches each); each core produces partial sums which
the host combines in float64.

Device-side strategy per core (8 batches):
  - all bulk tensors ship as fp8 (e4m3); the loss tolerates it easily
    (host-simulated rel err ~7e-4), halving HBM traffic vs bf16.
  - the mapping gather AND the ground-truth subtraction are fused into a
    single DoubleRow fp8 matmul per 512-column block: the 256-deep
    contraction holds [one-hot gather rows ; -I rows], so PSUM receives
    (x_gathered - gt) directly at 0.5 cycles/column.  The one-hot
    weights (an encoding of the integer `mapping` input, like the
    baseline's bf16 mapf) are host-packed with the data.
  - normalizer folds: masks x1/16 and landmark-point-columns x sqrt(2)
    are folded into the data; the auto-loss x16 (mean over D=512 vs the
    B*S divisor) is folded into the summation weights below; the best_*
    block is scaled so a single accumulator covers its three terms.
  - squares: 5 batches on ScalarE (Square -> fp8 tile), 3 on VectorE
    (PSUM->bf16 cast + bf16 self-multiply; only one PSUM operand is
    allowed per DVE op, and tensor_tensor_reduce faults on this
    runtime, both HW-verified).  The KL product q*ln(128q) runs on the
    otherwise-idle GpSimd.
  - ALL summation then happens on the TensorEngine: weight-vector
    matmuls ([16;1] pairs for fp8 DoubleRow tiles, 16/1 columns for the
    bf16 tiles) accumulate every squared tile into one PSUM bank -- row
    0 = main loss, row 32 = KL -- which one ScalarE copy extracts.
    This removes every per-batch accumulator read and DVE reduce.
"""

import contextlib
import ctypes
import os
import sys
import types

for _p in ("/opt/trn_rl_repo", "/root/.axon_site/_ro/trn_rl_repo"):
    if os.path.isdir(_p) and _p not in sys.path:
        sys.path.append(_p)

import numpy as np

# --- problem constants (hardcoded per spec) ---
B, S, N, D, V = 64, 128, 128, 512, 128
N_CORES = 8
BPC = B // N_CORES          # batches per core = 8
ALPHA, BETA, GAMMA, EPS = 1.0, 0.1, 1.0, 1e-20
MARK = (0, 29, 88, 117)
ACT_BATCHES = (3, 4, 5, 6, 7)   # squared via ScalarE -> fp8 jsq
DVE_BATCHES = (0, 1, 2)         # squared via VectorE cast+mult -> bf16 jsq

_CACHE = {}


def _install_ntff_hook_shim():
    """run_bass_kernel_spmd(trace=True) looks for antenv.axon_hooks, which
    this image lacks; recreate the ctypes hook against libaxon_pjrt.so."""
    if "antenv.axon_hooks" in sys.modules:
        return
    so_path = "/opt/axon/libaxon_pjrt.so"

    def _get_hook():
        if not os.path.exists(so_path):
            return None
        lib = ctypes.CDLL(so_path)
        if not hasattr(lib, "axon_start_nrt_profile"):
            return None
        lib.axon_start_nrt_profile.argtypes = [
            ctypes.POINTER(ctypes.c_int64), ctypes.c_size_t]
        lib.axon_start_nrt_profile.restype = ctypes.c_int64
        lib.axon_stop_nrt_profile.argtypes = [ctypes.c_char_p]
        lib.axon_stop_nrt_profile.restype = ctypes.c_int64

        @contextlib.contextmanager
        def _hook(output_dir, device_ids):
            import jax
            jax.devices()
            if device_ids:
                ids = (ctypes.c_int64 * len(device_ids))(*device_ids)
                rc = lib.axon_start_nrt_profile(ids, len(device_ids))
            else:
                rc = lib.axon_start_nrt_profile(None, 0)
            if rc != 0:
                raise RuntimeError(f"axon_start_nrt_profile rc={rc}")
            try:
                yield
            finally:
                n = lib.axon_stop_nrt_profile(str(output_dir).encode())
                if n < 0:
                    raise RuntimeError(f"axon_stop_nrt_profile rc={n}")

        return _hook

    mod = types.ModuleType("antenv.axon_hooks")
    mod.get_axon_ntff_profile_hook = _get_hook
    mod.set_axon_ntff_profile_hook = lambda h: None
    sys.modules["antenv.axon_hooks"] = mod


def _build_program():
    import concourse.bacc as bacc
    import concourse.tile as tile
    from concourse import mybir

    f32 = mybir.dt.float32
    bf16 = mybir.dt.bfloat16
    f8 = mybir.dt.float8e4
    nc = bacc.Bacc(None, target_bir_lowering=False, debug=False)

    # ---- per-core DRAM parameters (host pre-packed, all fp8) ----
    # data[s, b, 0:1024]  = [rzs | pts*wland | masks/16]      (gathered)
    # data[s, b, 1024:2048] = [zs | pts_gt*wland | masks_gt/16] (subtracted)
    d_data = nc.declare_dram_parameter("data", [S, BPC, 2048], f8, isOutput=False)
    # head[:, 0:16, :] = per-batch DoubleRow weights [E_b | -I]
    # head[:, 16:24, :] = qy (partition=s, per-batch 128 cols)
    # head[:, 24, :]   = best block: [brz|bp|bm] cols 0:64, [lg|bpg|bmg] 64:128
    d_head = nc.declare_dram_parameter("head", [128, 25, 128], f8, isOutput=False)
    # outputs
    d_orow = nc.declare_dram_parameter("o_row", [1, 1024], f32, isOutput=True)
    d_odve = nc.declare_dram_parameter("o_dve", [128, 1], f32, isOutput=True)

    SQUARE = mybir.ActivationFunctionType.Square
    COPY = mybir.ActivationFunctionType.Copy
    LN = mybir.ActivationFunctionType.Ln
    AL = mybir.AluOpType
    AX = mybir.AxisListType
    DR = mybir.MatmulPerfMode.DoubleRow

    with tile.TileContext(nc) as tc:
        with contextlib.ExitStack() as ctx:
            singles = ctx.enter_context(tc.tile_pool(name="singles", bufs=1))
            junkp = ctx.enter_context(tc.tile_pool(name="junk", bufs=2))
            psp = ctx.enter_context(tc.tile_pool(name="ps", bufs=3, space="PSUM"))
            rowp = ctx.enter_context(tc.tile_pool(name="row", bufs=1, space="PSUM"))

            # ---- SBUF tiles ----
            t_head = singles.tile([128, 25, 128], f8)
            chunks = [singles.tile([128, 2, 2, 1024], f8, tag=f"d{k}",
                                   name=f"chunk{k}")
                      for k in range(4)]
            t_lnq = singles.tile([128, 8, 128], bf16)
            t_eps = singles.tile([128, 1], f32)
            db = singles.tile([128, 64], bf16)
            a_dve = singles.tile([128, 1], f32)
            o_row = singles.tile([1, 1024], f32)
            # summation weight vectors (memset-built, all values fp8-exact)
            w16pair = singles.tile([128, 2, 128], f8)  # [16; 1] DoubleRow pair
            onepair = singles.tile([128, 2, 128], f8)  # [1; 1] DoubleRow pair
            w16col = singles.tile([128, 128], bf16)    # 16 (plain bf16 matmul)
            onecol = singles.tile([128, 128], bf16)    # 1

            # ---- input DMAs: head + 4 two-batch chunks ----
            nc.scalar.dma_start(out=chunks[0][:], in_=d_data.ap()[:, 0:2, :])
            nc.sync.dma_start(out=t_head[:], in_=d_head.ap())
            nc.sync.dma_start(out=chunks[1][:], in_=d_data.ap()[:, 2:4, :])
            nc.sync.dma_start(out=chunks[2][:], in_=d_data.ap()[:, 4:6, :])
            nc.sync.dma_start(out=chunks[3][:], in_=d_data.ap()[:, 6:8, :])

            nc.vector.memset(t_eps[:], EPS)
            nc.vector.memset(w16pair[:, 0, :], 16.0)
            nc.vector.memset(w16pair[:, 1, :], 1.0)
            nc.vector.memset(onepair[:], 1.0)
            nc.vector.memset(w16col[:], 16.0)
            nc.vector.memset(onecol[:], 1.0)

            # ---- gather+subtract matmuls: one DoubleRow pair per batch ----
            pss = []
            for b in range(BPC):
                ps = psp.tile([128, 1024], f32, tag="ps", name=f"ps{b}")
                pss.append(ps)
                wb = t_head[:, 2 * b:2 * b + 2, :]          # [128, 2, 128]
                rhs = chunks[b // 2][:, b % 2, :, :]        # [128, 2, 1024]
                nc.tensor.matmul(ps[:, 0:512], lhsT=wb, rhs=rhs[:, :, 0:512],
                                 start=True, stop=True, perf_mode=DR)
                nc.tensor.matmul(ps[:, 512:1024], lhsT=wb,
                                 rhs=rhs[:, :, 512:1024],
                                 start=True, stop=True, perf_mode=DR)

            # ---- ScalarE: Ln first (its table set also covers Square),
            # then five Square batches into fp8 jsq tiles ----
            qy_ap = t_head[:, 16:24, :]
            nc.scalar.activation(out=t_lnq[:], in_=qy_ap, func=LN,
                                 scale=float(V), bias=t_eps[:])
            jsq8 = {}
            for b in ACT_BATCHES:
                jq = junkp.tile([128, 2, 512], f8, tag="jq", name=f"jq{b}")
                jsq8[b] = jq
                nc.scalar.activation(out=jq[:], in_=pss[b][:], func=SQUARE)

            # ---- VectorE: best-block diff+square+reduce, PSUM->bf16 casts
            # and bf16 squares for its batches ----
            nc.vector.tensor_sub(db[:], t_head[:, 24, 0:64],
                                 t_head[:, 24, 64:128])
            jb = junkp.tile([128, 64], bf16, tag="jb")
            nc.vector.tensor_tensor(out=jb[:], in0=db[:], in1=db[:],
                                    op=AL.mult)
            nc.vector.tensor_reduce(out=a_dve[:, 0:1], in_=jb[:],
                                    axis=AX.X, op=AL.add)
            jsq16 = {}
            for b in DVE_BATCHES:
                jc = junkp.tile([128, 1024], bf16, tag="jc", name=f"jc{b}")
                nc.vector.tensor_copy(out=jc[:], in_=pss[b][:])
                jd = junkp.tile([128, 1024], bf16, tag="jd", name=f"jd{b}")
                jsq16[b] = jd
                nc.vector.tensor_tensor(out=jd[:], in0=jc[:], in1=jc[:],
                                        op=AL.mult)

            # ---- GpSimd: KL product q * ln(128 q) -> fp8 ----
            jk = junkp.tile([128, 2, 512], f8, tag="jk")
            nc.gpsimd.tensor_tensor(out=jk[:], in0=qy_ap, in1=t_lnq[:],
                                    op=AL.mult)

            # ---- TensorEngine summation into two PSUM row banks ----
            # row bank A accumulates main = sum(16*rz_diff^2 + pm_diff^2);
            # row bank B = sum of the KL products.
            rowt = rowp.tile([128, 512], f32, tag="rowm")
            rowk = rowp.tile([128, 512], f32, tag="rowk")
            nc.tensor.matmul(rowk[:], lhsT=onepair[:], rhs=jk[:],
                             start=True, stop=True, perf_mode=DR)
            first = True
            for b in (0, 3, 1, 4, 2, 5, 6, 7):
                last = b == 7
                if b in jsq8:
                    nc.tensor.matmul(rowt[:], lhsT=w16pair[:],
                                     rhs=jsq8[b][:], start=first, stop=last,
                                     perf_mode=DR, skip_group_check=True)
                else:
                    jd = jsq16[b]
                    nc.tensor.matmul(rowt[:], lhsT=w16col[:],
                                     rhs=jd[:, 0:512], start=first,
                                     stop=False, skip_group_check=True)
                    nc.tensor.matmul(rowt[:], lhsT=onecol[:],
                                     rhs=jd[:, 512:1024], start=False,
                                     stop=last, skip_group_check=True)
                first = False

            # ---- extract rows + store ----
            nc.scalar.activation(out=o_row[:, 0:512], in_=rowt[0:1, :],
                                 func=COPY)
            nc.vector.tensor_copy(out=o_row[:, 512:1024], in_=rowk[0:1, :])
            nc.sync.dma_start(out=d_orow.ap(), in_=o_row[:])
            nc.sync.dma_start(out=d_odve.ap(), in_=a_dve[:])

    nc.compile()
    return nc


def _get_program():
    if "nc" not in _CACHE:
        _CACHE["nc"] = _build_program()
    return _CACHE["nc"]


def _shard_inputs(inputs):
    """Pack the full B=64 inputs into 8 per-core fp8 input maps."""
    import ml_dtypes
    f8 = ml_dtypes.float8_e4m3
    f = lambda k: np.asarray(inputs[k], dtype=np.float32)

    wland = np.ones(N, dtype=np.float32)
    for n in MARK:
        wland[n] = np.sqrt(np.float32(2.0))

    # gathered block  [B,S,1024] and subtracted block [B,S,1024]
    pts = f("pts") * wland[None, None, :, None]
    ptsg = f("pts_gt") * wland[None, None, :, None]
    xg = np.concatenate([f("rzs"),
                         pts.reshape(B, S, 2 * N),
                         f("masks").reshape(B, S, 2 * N) * np.float32(1 / 16)],
                        axis=2)
    gt = np.concatenate([f("zs"),
                         ptsg.reshape(B, S, 2 * N),
                         f("masks_gt").reshape(B, S, 2 * N) * np.float32(1 / 16)],
                        axis=2)
    data = np.concatenate([xg, gt], axis=2).astype(f8)      # [B,S,2048]

    mapping = np.asarray(inputs["mapping"]).astype(np.int64)  # [B,S]
    iota = np.arange(128, dtype=np.int64)
    # one-hot E_b[p, m] = (mapping[b, m] == p), negI[p, m] = -(p == m)
    onehot = (mapping[:, None, :] == iota[None, :, None]).astype(f8)  # [B,128,128]
    negi = (-np.eye(128, dtype=np.float32)).astype(f8)

    qy = f("qy")

    # best block, scaled so one accumulator (div B*D) covers all 3 terms:
    #   auto: x1; pt: *wbest/64 (folds best_landmark and the /(2 B^2 N^2));
    #   seg: slice-zeroed, x2 (folds /(128 B)).
    wbest = np.ones(N, dtype=np.float32)
    rb = np.sqrt(np.float32(1.0 + 2.0 * B * N))
    for n in MARK:
        wbest[n] = rb
    bp = f("best_pt") * wbest[None, :, None] * np.float32(1 / 64)
    bpg = f("best_pt_gt") * wbest[None, :, None] * np.float32(1 / 64)
    bm = f("best_mask").copy() * np.float32(2.0)
    bmg = f("best_mask_gt").copy() * np.float32(2.0)
    bm[:, :32] = 0.0
    bm[:, 96:] = 0.0
    bmg[:, :32] = 0.0
    bmg[:, 96:] = 0.0
    brz = f("best_rz")
    lg = f("logits")

    in_maps = []
    for c in range(N_CORES):
        lo, hi = c * BPC, (c + 1) * BPC
        head = np.zeros((128, 25, 128), dtype=f8)
        head[:, 0:16:2, :] = onehot[lo:hi].transpose(1, 0, 2)
        head[:, 1:16:2, :] = negi[:, None, :]
        head[:, 16:24, :] = qy[lo:hi].transpose(1, 0, 2).astype(f8)
        half0 = np.concatenate([brz[lo:hi].reshape(128, 32),
                                bp[lo:hi].reshape(128, 16),
                                bm[lo:hi].reshape(128, 16)], axis=1)
        half1 = np.concatenate([lg[lo:hi].reshape(128, 32),
                                bpg[lo:hi].reshape(128, 16),
                                bmg[lo:hi].reshape(128, 16)], axis=1)
        head[:, 24, 0:64] = half0.astype(f8)
        head[:, 24, 64:128] = half1.astype(f8)
        m = {
            "data": np.ascontiguousarray(data[lo:hi].transpose(1, 0, 2)),
            "head": head,
        }
        in_maps.append(m)
    return in_maps


def _combine(results, ln_v):
    """Host-side float64 reduction of the per-core partial sums."""
    s_main = s_kld = s_best = 0.0
    for r in results:
        orow = r["o_row"].astype(np.float64)
        s_main += orow[0, 0:512].sum()
        s_kld += orow[0, 512:1024].sum()
        s_best += r["o_dve"].astype(np.float64).sum()
    # device computed sum q*ln(128 q); only valid for vector_dims == 128
    if abs(ln_v - np.log(128.0)) > 1e-12:
        raise ValueError("kernel compiled for vector_dims == 128")

    main = s_main / (B * S)
    kld = s_kld / (B * S)
    best = s_best / (B * D)
    ret = best + main + BETA * kld
    return np.float32(ret * B)


def run_sharded(inputs, trace=False):
    """Compile (cached), run on the 8 cores, return (scalar, BassKernelResults)."""
    _install_ntff_hook_shim()
    from concourse.bass_utils import run_bass_kernel_spmd

    ln_v = float(np.log(float(inputs["vector_dims"])))
    nc = _get_program()
    in_maps = _shard_inputs(inputs)
    res = run_bass_kernel_spmd(nc, in_maps, list(range(N_CORES)), trace=trace)
    return _combine(res.results, ln_v), res


def kernel(**inputs) -> np.ndarray:
    out, _ = run_sharded(inputs, trace=False)
    return out
